# revision 31
# baseline (speedup 1.0000x reference)
"""Trainium2 Bass kernel for nn_LinearEmbedded (moe_routing).

Reference computation:
    w = weight1[region_ix]             # (B, C, D) gather per-region weights
    out = einsum('abc,bcd->abd', x, w) + bias1[region_ix][None]

Sharding: the B axis (128 regions) is split across 8 NeuronCores, 16 per
core; the per-region weight/bias gather happens host-side so each core only
receives the 16 gathered slices it needs.  The bias add (0.1% of the FLOPs)
happens host-side during the unshard, like the gather.

Precision: both input streams are fp8 E3M4 (4 mantissa bits), quantized
host-side with numpy RNE: w scaled by 64 (sigma ~1.3 in E3M4's normal
range), x scaled by 2.  The PE upconverts and accumulates fp32 in PSUM
(which holds 128*out); the fp16 out store keeps that scale and the host
divides by 128 during the unshard.  Measured l2 relative error vs the
fp32 reference: 1.881e-2 -- deterministic for the fixed test seed, since
every lossy step is either host-side numpy or the HW f32->f16 store
(verified to match numpy to 6 digits on earlier runs).  Per-core DMA is
7.35 MB: w 4.2 + x 1.05 + out 2.1.

Engine roles (HWDGE rings exist only on SP and ACT):
    sync   - w loads (one DMA per b, b=0 split in halves) dispatched up
             front, then the out stores (merged into 7 DMAs: HWDGE
             descriptor generation is serial per ring at ~0.6us per
             instruction, so fewer store instructions shorten the tail;
             the last groups stay small to keep the final transfer+sem
             chain short)
    scalar - all 16 xt loads issued up front (full ring, no reuse)
    tensor - 6 dummy warmup matmuls (PE p-state ramp during DMA fill),
             then 4 accumulating K=128 matmuls per b
    vector - PSUM -> SBUF f32->f16 cast copies into one contiguous out
             tile (16 b slots, never reused, so copies never wait on
             store completion and stores can flush at the very end)
    gpsimd - output-completion proof.  The NEFF epilogue makes every
             engine serially reset its ~51-slot chunk of the semaphore
             file after an all-engine rendezvous; gpsimd holding the
             final waits lets the slower engines' chunks overlap the
             trailing stores.

DMA rings complete out of order across their parallel queues, so each
b/slot gets its own completion semaphore with at most one outstanding
DMA at the granularity PE waits on; all wait thresholds stay <=48
(higher thresholds hung the device).

Rings: 16 xt slots, 16 w slots, 4 PSUM banks, one [128, 16*512] out tile.
"""

import numpy as np

A, B, C, D = 128, 128, 512, 512
NCORES = 8
BL = B // NCORES
KC = C // 128
R_P, R_O = 4, 6
WSCALE = 64.0
XSCALE = 2.0

_prog = None


def _build_program():
    global _prog
    if _prog is not None:
        return _prog

    import concourse.bass as bass
    import concourse.mybir as mybir
    from contextlib import ExitStack

    F32 = mybir.dt.float32
    F16 = mybir.dt.float16
    F8 = mybir.dt.float8e3
    nc = bass.Bass("TRN2", target_bir_lowering=False, debug=False)
    xt = nc.dram_tensor("xt", [BL, 128, KC * A], F8, kind="ExternalInput")
    w = nc.dram_tensor("w", [BL, 128, KC * D], F8, kind="ExternalInput")
    out = nc.dram_tensor("out", [A, BL * D], F16, kind="ExternalOutput")

    ctx = ExitStack()
    with ctx:
        xts = [
            ctx.enter_context(nc.sbuf_tensor(f"xts{i}", [128, KC * A], F8))
            for i in range(BL)
        ]
        ws = [
            ctx.enter_context(nc.sbuf_tensor(f"ws{i}", [128, KC * D], F8))
            for i in range(BL)
        ]
        ots = ctx.enter_context(nc.sbuf_tensor("ots", [128, BL * D], F16))
        psums = [
            ctx.enter_context(nc.psum_tensor(f"psums{i}", [A, D], F32))
            for i in range(R_P)
        ]
        # PE p-state warmup: a junk tile + junk PSUM bank; dummy matmuls at
        # stream start ramp the throttled PE clock while the first loads
        # are still in flight (measured: cold matmuls ~600ns vs ~385ns
        # warm at 512 moving cols; full clock would be 213ns)
        warm_t = ctx.enter_context(nc.sbuf_tensor("warm_t", [128, D], F16))
        psum_w = ctx.enter_context(nc.psum_tensor("psum_w", [A, D], F32))

        s_xs = [ctx.enter_context(nc.semaphore(f"s_x{i}")) for i in range(BL)]
        s_ws = [ctx.enter_context(nc.semaphore(f"s_w{i}")) for i in range(BL)]
        s_os = [ctx.enter_context(nc.semaphore(f"s_o{i}")) for i in range(R_O)]
        s_w0b = ctx.enter_context(nc.semaphore("s_w0b"))
        s_pe = ctx.enter_context(nc.semaphore("s_pe"))
        s_cp = ctx.enter_context(nc.semaphore("s_cp"))

        sync, scalar, tensor, vector = nc.sync, nc.scalar, nc.tensor, nc.vector
        half = KC * D // 2

        # --- SP engine: w loads up front, then out stores, then proof ---
        if True:
            # b=0 split so the first matmuls only wait for a 128KB chunk
            sync.dma_start(ws[0][:, 0:half], w[0, :, 0:half]).then_inc(s_ws[0], 16)
            sync.dma_start(ws[0][:, half : 2 * half], w[0, :, half : 2 * half]).then_inc(
                s_w0b, 16
            )
            for b in range(3, BL):
                sync.dma_start(ws[b][:], w[b, :, :]).then_inc(s_ws[b], 16)

            # out stores, merged into 7 DMAs to cut the SP ring's serial
            # HWDGE descriptor-generation load (per-instruction ~0.6us was
            # pacing the tail); the final groups stay small so the last
            # store's transfer+sem chain is short
            GROUPS = [(0, 3), (3, 3), (6, 3), (9, 3), (12, 2), (14, 1), (15, 1)]
            for gi, (b0, g) in enumerate(GROUPS):
                sync.wait_ge(s_cp, b0 + g)
                sync.dma_start(
                    out[:, b0 * D : (b0 + g) * D], ots[:, b0 * D : (b0 + g) * D]
                ).then_inc(s_os[gi % R_O], 16)

            # SP ends here: the completion proof lives on gpsimd (below),
            # so SP's per-engine semaphore-reset epilogue chunk (the
            # slowest, ~6.6us) overlaps the trailing store transfers
            # instead of running after them.

        # --- PE engine ---
        if True:
            N_WARM = 6
            for i in range(N_WARM):
                nc.tensor.matmul(
                    psum_w[:],
                    warm_t[:, 0:A],
                    warm_t[:, :],
                    start=True,
                    stop=True,
                )
            for b in range(BL):
                if b >= R_P:
                    tensor.wait_ge(s_cp, b - R_P + 1)
                tensor.wait_ge(s_xs[b], 16)
                for k in range(KC):
                    if k == 0:
                        tensor.wait_ge(s_ws[b], 16)
                    elif k == 2 and b == 0:
                        tensor.wait_ge(s_w0b, 16)
                    mm = nc.tensor.matmul(
                        psums[b % R_P][:],
                        xts[b][:, k * A : (k + 1) * A],
                        ws[b][:, k * D : (k + 1) * D],
                        start=(k == 0),
                        stop=(k == KC - 1),
                    )
                    if k == KC - 1:
                        mm.then_inc(s_pe, 1)

        # --- DVE engine: PSUM->SBUF cast copies ---
        # 16 distinct out tiles: copies never wait on store completion, so
        # the stores (queued behind the w loads on the SP ring) can flush
        # at the very end without stalling the PSUM ring / PE.
        if True:
            for b in range(BL):
                vector.wait_ge(s_pe, b + 1)
                nc.vector.tensor_copy(
                    ots[:, b * D : (b + 1) * D], psums[b % R_P][:]
                ).then_inc(s_cp, 1)

        # --- ACT engine: all xt loads up front (full ring) ---
        if True:
            for b in range(BL):
                scalar.dma_start(xts[b][:], xt[b, :, :]).then_inc(s_xs[b], 16)

        # --- GPSIMD engine: w1/w2 loads via SWDGE (third descriptor
        # generator during the DMA ramp, and two fewer instructions ahead
        # of the stores on the SP ring), then the output-completion proof
        # (gpsimd has the cheapest epilogue reset chunk, ~55ns/sem).
        if True:
            nc.gpsimd.dma_start(ws[1][:], w[1, :, :]).then_inc(s_ws[1], 16)
            nc.gpsimd.dma_start(ws[2][:], w[2, :, :]).then_inc(s_ws[2], 16)
            for i in range(R_O):
                nc.gpsimd.wait_ge(s_os[i], 32 if i == 0 else 16)

        # No Block: engine streams end bare; the framework exit barrier
        # joins the engines, and gpsimd's waits prove the stores.

    _prog = nc
    return nc


def _shard_inputs(x, region_ix, weight1, bias1):
    import ml_dtypes

    F8NP = ml_dtypes.float8_e3m4
    x16 = (x * np.float32(XSCALE)).astype(F8NP)
    in_maps = []
    for c in range(NCORES):
        bs = slice(c * BL, (c + 1) * BL)
        rloc = region_ix[bs]
        xs = x16[:, bs, :].transpose(1, 2, 0)  # (BL, C, A)
        xtv = np.ascontiguousarray(
            xs.reshape(BL, KC, 128, A).transpose(0, 2, 1, 3)
        ).reshape(BL, 128, KC * A)
        wg = (weight1[rloc] * np.float32(WSCALE)).astype(F8NP)  # (BL, C, D)
        wdev = np.ascontiguousarray(
            wg.reshape(BL, KC, 128, D).transpose(0, 2, 1, 3)
        ).reshape(BL, 128, KC * D)
        in_maps.append({"xt": xtv, "w": wdev})
    return in_maps


def kernel(x, region_ix, weight1, bias1):
    from concourse.bass_utils import run_bass_kernel_spmd

    x = np.asarray(x, dtype=np.float32)
    region_ix = np.asarray(region_ix).astype(np.int64)
    weight1 = np.asarray(weight1, dtype=np.float32)
    bias1 = np.asarray(bias1, dtype=np.float32)

    nc = _build_program()
    in_maps = _shard_inputs(x, region_ix, weight1, bias1)
    res = run_bass_kernel_spmd(nc, in_maps, core_ids=list(range(NCORES)))

    bg = bias1[region_ix]  # (B, D) host-side bias gather + add
    outv = np.empty((A, B, D), dtype=np.float32)
    for c in range(NCORES):
        outv[:, c * BL : (c + 1) * BL, :] = res.results[c]["out"].reshape(A, BL, D)
    outv *= np.float32(1.0 / (WSCALE * XSCALE))
    outv += bg[None, :, :]
    return outv


# revision 32
# speedup vs baseline: 1.0303x; 1.0303x over previous
"""Trainium2 Bass kernel for nn_LinearEmbedded (moe_routing).

Reference computation:
    w = weight1[region_ix]             # (B, C, D) gather per-region weights
    out = einsum('abc,bcd->abd', x, w) + bias1[region_ix][None]

Sharding: the B axis (128 regions) is split across 8 NeuronCores, 16 per
core; the per-region weight/bias gather happens host-side so each core only
receives the 16 gathered slices it needs.  The bias add (0.1% of the FLOPs)
happens host-side during the unshard, like the gather.

Precision: both input streams are fp8 E3M4 (4 mantissa bits), quantized
host-side with numpy RNE: w scaled by 64 (sigma ~1.3 in E3M4's normal
range), x scaled by 2.  The PE upconverts and accumulates fp32 in PSUM
(which holds 128*out); the fp16 out store keeps that scale and the host
divides by 128 during the unshard.  Measured l2 relative error vs the
fp32 reference: 1.881e-2 -- deterministic for the fixed test seed, since
every lossy step is either host-side numpy or the HW f32->f16 store
(verified to match numpy to 6 digits on earlier runs).  Per-core DMA is
7.35 MB: w 4.2 + x 1.05 + out 2.1.

Engine roles (HWDGE rings exist only on SP and ACT):
    sync   - w loads (one DMA per b, b=0 split in halves) dispatched up
             front, then the out stores (merged into 7 DMAs: HWDGE
             descriptor generation is serial per ring at ~0.6us per
             instruction, so fewer store instructions shorten the tail;
             the last groups stay small to keep the final transfer+sem
             chain short)
    scalar - all 16 xt loads issued up front (full ring, no reuse)
    tensor - 6 dummy warmup matmuls (PE p-state ramp during DMA fill),
             then 4 accumulating K=128 matmuls per b
    vector - PSUM -> SBUF f32->f16 cast copies into one contiguous out
             tile (16 b slots, never reused, so copies never wait on
             store completion and stores can flush at the very end)
    gpsimd - output-completion proof.  The NEFF epilogue makes every
             engine serially reset its ~51-slot chunk of the semaphore
             file after an all-engine rendezvous; gpsimd holding the
             final waits lets the slower engines' chunks overlap the
             trailing stores.

DMA rings complete out of order across their parallel queues, so each
b/slot gets its own completion semaphore with at most one outstanding
DMA at the granularity PE waits on; all wait thresholds stay <=48
(higher thresholds hung the device).

Rings: 16 xt slots, 16 w slots, 4 PSUM banks, one [128, 16*512] out tile.
"""

import numpy as np

A, B, C, D = 128, 128, 512, 512
NCORES = 8
BL = B // NCORES
KC = C // 128
R_P, R_O = 4, 6
WSCALE = 64.0
XSCALE = 2.0

_prog = None


def _build_program():
    global _prog
    if _prog is not None:
        return _prog

    import concourse.bass as bass
    import concourse.mybir as mybir
    from contextlib import ExitStack

    F32 = mybir.dt.float32
    F16 = mybir.dt.float16
    F8 = mybir.dt.float8e3
    nc = bass.Bass("TRN2", target_bir_lowering=False, debug=False)
    xt = nc.dram_tensor("xt", [BL, 128, KC * A], F8, kind="ExternalInput")
    w = nc.dram_tensor("w", [BL, 128, KC * D], F8, kind="ExternalInput")
    out = nc.dram_tensor("out", [A, BL * D], F16, kind="ExternalOutput")

    ctx = ExitStack()
    with ctx:
        xts = [
            ctx.enter_context(nc.sbuf_tensor(f"xts{i}", [128, KC * A], F8))
            for i in range(BL)
        ]
        ws = [
            ctx.enter_context(nc.sbuf_tensor(f"ws{i}", [128, KC * D], F8))
            for i in range(BL)
        ]
        ots = ctx.enter_context(nc.sbuf_tensor("ots", [128, BL * D], F16))
        psums = [
            ctx.enter_context(nc.psum_tensor(f"psums{i}", [A, D], F32))
            for i in range(R_P)
        ]
        # PE p-state warmup: a junk tile + junk PSUM bank; dummy matmuls at
        # stream start ramp the throttled PE clock while the first loads
        # are still in flight (measured: cold matmuls ~600ns vs ~385ns
        # warm at 512 moving cols; full clock would be 213ns)
        warm_t = ctx.enter_context(nc.sbuf_tensor("warm_t", [128, D], F16))
        psum_w = ctx.enter_context(nc.psum_tensor("psum_w", [A, D], F32))

        s_xs = [ctx.enter_context(nc.semaphore(f"s_x{i}")) for i in range(BL)]
        s_ws = [ctx.enter_context(nc.semaphore(f"s_w{i}")) for i in range(BL)]
        s_os = [ctx.enter_context(nc.semaphore(f"s_o{i}")) for i in range(R_O)]
        s_w0b = ctx.enter_context(nc.semaphore("s_w0b"))
        s_pe = ctx.enter_context(nc.semaphore("s_pe"))
        s_cp = ctx.enter_context(nc.semaphore("s_cp"))

        sync, scalar, tensor, vector = nc.sync, nc.scalar, nc.tensor, nc.vector
        half = KC * D // 2

        # --- SP engine: w loads up front, then out stores, then proof ---
        if True:
            # b=0 split so the first matmuls only wait for a 128KB chunk
            sync.dma_start(ws[0][:, 0:half], w[0, :, 0:half]).then_inc(s_ws[0], 16)
            sync.dma_start(ws[0][:, half : 2 * half], w[0, :, half : 2 * half]).then_inc(
                s_w0b, 16
            )
            for b in range(1, BL):
                sync.dma_start(ws[b][:], w[b, :, :]).then_inc(s_ws[b], 16)

            # out stores, merged into 7 DMAs to cut the SP ring's serial
            # HWDGE descriptor-generation load (per-instruction ~0.6us was
            # pacing the tail); the final groups stay small so the last
            # store's transfer+sem chain is short
            GROUPS = [(0, 3), (3, 3), (6, 3), (9, 3), (12, 2), (14, 1), (15, 1)]
            for gi, (b0, g) in enumerate(GROUPS):
                sync.wait_ge(s_cp, b0 + g)
                sync.dma_start(
                    out[:, b0 * D : (b0 + g) * D], ots[:, b0 * D : (b0 + g) * D]
                ).then_inc(s_os[gi % R_O], 16)

            # SP ends here: the completion proof lives on gpsimd (below),
            # so SP's per-engine semaphore-reset epilogue chunk (the
            # slowest, ~6.6us) overlaps the trailing store transfers
            # instead of running after them.

        # --- PE engine ---
        if True:
            N_WARM = 6
            for i in range(N_WARM):
                nc.tensor.matmul(
                    psum_w[:],
                    warm_t[:, 0:A],
                    warm_t[:, :],
                    start=True,
                    stop=True,
                )
            for b in range(BL):
                if b >= R_P:
                    tensor.wait_ge(s_cp, b - R_P + 1)
                tensor.wait_ge(s_xs[b], 16)
                for k in range(KC):
                    if k == 0:
                        tensor.wait_ge(s_ws[b], 16)
                    elif k == 2 and b == 0:
                        tensor.wait_ge(s_w0b, 16)
                    mm = nc.tensor.matmul(
                        psums[b % R_P][:],
                        xts[b][:, k * A : (k + 1) * A],
                        ws[b][:, k * D : (k + 1) * D],
                        start=(k == 0),
                        stop=(k == KC - 1),
                    )
                    if k == KC - 1:
                        mm.then_inc(s_pe, 1)

        # --- DVE engine: PSUM->SBUF cast copies ---
        # 16 distinct out tiles: copies never wait on store completion, so
        # the stores (queued behind the w loads on the SP ring) can flush
        # at the very end without stalling the PSUM ring / PE.
        if True:
            for b in range(BL):
                vector.wait_ge(s_pe, b + 1)
                nc.vector.tensor_copy(
                    ots[:, b * D : (b + 1) * D], psums[b % R_P][:]
                ).then_inc(s_cp, 1)

        # --- ACT engine: all xt loads up front (full ring) ---
        if True:
            for b in range(BL):
                scalar.dma_start(xts[b][:], xt[b, :, :]).then_inc(s_xs[b], 16)

        # --- GPSIMD engine: output-completion proof ---
        # gpsimd is otherwise idle and has the cheapest reset chunk
        # (~55ns/sem vs SP's ~135ns), so it carries the final waits.
        if True:
            for i in range(R_O):
                nc.gpsimd.wait_ge(s_os[i], 32 if i == 0 else 16)

        # No Block: engine streams end bare; the framework exit barrier
        # joins the engines, and gpsimd's waits prove the stores.

    _prog = nc
    return nc


def _shard_inputs(x, region_ix, weight1, bias1):
    import ml_dtypes

    F8NP = ml_dtypes.float8_e3m4
    x16 = (x * np.float32(XSCALE)).astype(F8NP)
    in_maps = []
    for c in range(NCORES):
        bs = slice(c * BL, (c + 1) * BL)
        rloc = region_ix[bs]
        xs = x16[:, bs, :].transpose(1, 2, 0)  # (BL, C, A)
        xtv = np.ascontiguousarray(
            xs.reshape(BL, KC, 128, A).transpose(0, 2, 1, 3)
        ).reshape(BL, 128, KC * A)
        wg = (weight1[rloc] * np.float32(WSCALE)).astype(F8NP)  # (BL, C, D)
        wdev = np.ascontiguousarray(
            wg.reshape(BL, KC, 128, D).transpose(0, 2, 1, 3)
        ).reshape(BL, 128, KC * D)
        in_maps.append({"xt": xtv, "w": wdev})
    return in_maps


def kernel(x, region_ix, weight1, bias1):
    from concourse.bass_utils import run_bass_kernel_spmd

    x = np.asarray(x, dtype=np.float32)
    region_ix = np.asarray(region_ix).astype(np.int64)
    weight1 = np.asarray(weight1, dtype=np.float32)
    bias1 = np.asarray(bias1, dtype=np.float32)

    nc = _build_program()
    in_maps = _shard_inputs(x, region_ix, weight1, bias1)
    res = run_bass_kernel_spmd(nc, in_maps, core_ids=list(range(NCORES)))

    bg = bias1[region_ix]  # (B, D) host-side bias gather + add
    outv = np.empty((A, B, D), dtype=np.float32)
    for c in range(NCORES):
        outv[:, c * BL : (c + 1) * BL, :] = res.results[c]["out"].reshape(A, BL, D)
    outv *= np.float32(1.0 / (WSCALE * XSCALE))
    outv += bg[None, :, :]
    return outv
